# revision 6
# baseline (speedup 1.0000x reference)
"""Trainium2 Bass kernel for nn_MultiHeadAttention_60816736911814.

Reference semantics (all derived from `src`; `k`/`v` args ignored):
  x  = channel_shuffle(src)          # [B,S,G,C]->[B,S,C,G] flatten, G=5
  xh = split_heads(x)                # [B,H,S,dk], H=16, dk=80
  q/k/v = per-head Linear(dk,dk)     # weights [H,dk,dk] + bias
  attn  = softmax(q kᵀ / sqrt(dk)) v
  out   = concat(attn) @ Woᵀ + bo    # Wo [D,D], D=1280

Sharding (8 cores, no collectives): core i handles batch b=i//2 and query
rows [512*(i%2), 512*(i%2)+512). Each core gets src[b] ROLLED so its query
rows are rows 0..511 (key order is irrelevant to softmax+sum), letting all
cores run an identical program. Wo is applied per-core on its row slice, so
the full output is a pure concatenation.

All matmuls run in bf16 with fp32 PSUM accumulation. The channel shuffle,
head split and Linear biases are folded into host-side weight layout:
 - device-side xhT rows use d' ordering with d = 5*(d'%16) + d'//16, so the
   channels of head h at row d' are exactly src channel 256*(d'//16)+16h+
   (d'%16) -> a contiguous 16-channel strip per (h, r=d'//16), produced by
   plain 128x128 PE transposes of src + one rectangular SBUF->SBUF DMA.
 - projection weights are permuted with the same d' order and get the bias
   appended as contraction row 80 (paired with a ones row 80 in xhT).
 - softmax denominator Z comes free as row 80 of the attention matmul by
   augmenting V with a ones column.
"""

import numpy as np
import ml_dtypes

B, S, D = 4, 1024, 1280
H, DK, G = 16, 80, 5
N_CORES = 8
SH = S // 2  # 512 query rows per core
SCALE = 1.0 / float(np.sqrt(DK))
NT = S // 128  # 8 s-tiles
NCT = D // 128  # 10 channel tiles

_BUILT = {}


def _legalize_waits(nc, mybir):
    """This walrus build allows 1 sync-wait per instruction (2 on
    EventSemaphore). Tile can emit more; split overflow waits onto
    injected same-engine NoOp carriers placed just before the
    instruction (engines run their stream in order -> AND semantics)."""
    n_fix = 0
    for f in nc.m.functions:
        for blk in f.blocks:
            out = []
            changed = False
            for inst in blk.instructions:
                cap = 2 if type(inst).__name__ == "InstEventSemaphore" else 1
                si = inst.sync_info
                if si is not None and si.on_wait and len(si.on_wait) > cap:
                    waits = list(si.on_wait)
                    for w in waits[:-cap]:
                        nop = mybir.InstNoOp(name=f"I-waitfix-{n_fix}")
                        n_fix += 1
                        nop.engine = inst.engine
                        nop.sync_info = mybir.SyncInfo(on_wait=[w], on_update=[])
                        out.append(nop)
                    inst.sync_info = mybir.SyncInfo(
                        on_wait=waits[-cap:], on_update=list(si.on_update)
                    )
                    changed = True
                out.append(inst)
            if changed:
                try:
                    blk.instructions = out
                except Exception:
                    blk.instructions.clear()
                    blk.instructions.extend(out)
    return n_fix


def _build():
    import concourse.bass as bass
    import concourse.mybir as mybir
    import concourse.tile as tile

    f32 = mybir.dt.float32
    bf16 = mybir.dt.bfloat16

    nc = bass.Bass(trn_type="TRN2", target_bir_lowering=False, debug=False)

    x_d = nc.dram_tensor("x", [S, D], f32, kind="ExternalInput").ap()
    wq_d = nc.dram_tensor("wq", [DK + 1, H, DK], bf16, kind="ExternalInput").ap()
    wk_d = nc.dram_tensor("wk", [DK + 1, H, DK], bf16, kind="ExternalInput").ap()
    wv_d = nc.dram_tensor("wv", [DK + 1, H, DK], bf16, kind="ExternalInput").ap()
    wo_d = nc.dram_tensor("wo", [DK + 1, H, D], bf16, kind="ExternalInput").ap()
    id_d = nc.dram_tensor("ident", [128, 128], bf16, kind="ExternalInput").ap()
    on2_d = nc.dram_tensor("ones2d", [128, 128], bf16, kind="ExternalInput").ap()
    onr_d = nc.dram_tensor("onesrow", [1, H * S], bf16, kind="ExternalInput").ap()
    on80_d = nc.dram_tensor("ones80", [1, DK], bf16, kind="ExternalInput").ap()
    out_d = nc.dram_tensor("out", [SH, D], f32, kind="ExternalOutput").ap()

    with tile.TileContext(nc) as tc:
        with (
            tc.tile_pool(name="const", bufs=1) as const,
            tc.tile_pool(name="big", bufs=1) as big,
            tc.tile_pool(name="ld", bufs=3) as ld,
            tc.tile_pool(name="et", bufs=4) as etp,
            tc.tile_pool(name="sm", bufs=2) as sm,
            tc.tile_pool(name="ps", bufs=7, space="PSUM") as ps,
        ):
            ident = const.tile([128, 128], bf16)
            nc.sync.dma_start(out=ident, in_=id_d)
            ones80 = const.tile([1, DK], bf16)
            nc.sync.dma_start(out=ones80, in_=on80_d)

            wq_sb = big.tile([DK + 1, H, DK], bf16)
            wk_sb = big.tile([DK + 1, H, DK], bf16)
            wv_sb = big.tile([DK + 1, H, DK], bf16)
            wo_sb = big.tile([DK + 1, H, D], bf16)
            nc.sync.dma_start(out=wq_sb, in_=wq_d)
            nc.sync.dma_start(out=wk_sb, in_=wk_d)
            nc.sync.dma_start(out=wv_sb, in_=wv_d)
            nc.sync.dma_start(out=wo_sb, in_=wo_d)

            # XH[d', h, s]: transposed shuffled heads (+ ones row 80)
            xh = big.tile([DK + 1, H, S], bf16)
            nc.sync.dma_start(out=xh[DK : DK + 1, :, :], in_=onr_d)
            # V_ALL[s_in_tile, t*16+h, e(+pad, ones at 96)]
            VW = 97  # Z lands on PSUM partition 96 (32-aligned for engine reads)
            vall = big.tile([128, NT * H, VW], bf16)
            on_bcast = bass.AP(
                tensor=on2_d.tensor,
                offset=0,
                ap=[[128, 128], [0, NT * H], [1, VW - DK]],
            )
            nc.sync.dma_start(out=vall[:, :, DK:VW], in_=on_bcast)
            # concatT[e(+ones), h, q]
            ct = big.tile([DK + 1, H, SH], bf16)
            nc.sync.dma_start(out=ct[DK : DK + 1, 15, :], in_=onr_d[:, 0:SH])

            # ---- Stage 1: load src, cast, transpose, repack ----
            xt = big.tile([128, NCT, S], bf16)  # x transposed [c, ct, s]
            for t in range(NT):
                s_f = ld.tile([128, D], f32)
                nc.sync.dma_start(out=s_f, in_=x_d[t * 128 : (t + 1) * 128, :])
                s_b = ld.tile([128, D], bf16)
                nc.vector.tensor_copy(s_b, s_f)
                for c in range(NCT):
                    p_ps = ps.tile([128, 128], bf16, tag="ps")
                    nc.tensor.transpose(p_ps, s_b[:, c * 128 : (c + 1) * 128], ident)
                    nc.scalar.copy(xt[:, c, t * 128 : (t + 1) * 128], p_ps)
            for h in range(H):
                for r in range(5):
                    c = 2 * r + h // 8
                    poff = 16 * (h % 8)
                    nc.sync.dma_start(
                        out=xh[16 * r : 16 * r + 16, h, :],
                        in_=xt[poff : poff + 16, c, :],
                    )

            # ---- Stage 3: V projections (t-outer, head-grouped) ----
            groups = [list(range(0, 6)), list(range(6, 12)), list(range(12, 16))]
            for t in range(NT):
                for grp in groups:
                    ng = len(grp)
                    vp = ps.tile([128, 6, DK], f32, tag="ps")
                    for i, h in enumerate(grp):
                        nc.tensor.matmul(
                            vp[:, i, :],
                            xh[:, h, t * 128 : (t + 1) * 128],
                            wv_sb[:, h, :],
                            start=True,
                            stop=True,
                        )
                    nc.vector.tensor_copy(
                        vall[:, t * H + grp[0] : t * H + grp[0] + ng, 0:DK],
                        vp[:, 0:ng, :],
                    )

            # ---- Stage 4: projections + attention per head ----
            for h in range(H):
                qt_ps = ps.tile([DK, SH], f32, tag="ps")
                nc.tensor.matmul(
                    qt_ps, wq_sb[:, h, :], xh[:, h, 0:SH], start=True, stop=True
                )
                qt_sb = sm.tile([DK, SH], bf16, tag="qt")
                nc.vector.tensor_copy(qt_sb, qt_ps)
                kt_sb = sm.tile([DK, S], bf16, tag="kt")
                for j in range(2):
                    kt_ps = ps.tile([DK, SH], f32, tag="ps")
                    nc.tensor.matmul(
                        kt_ps,
                        wk_sb[:, h, :],
                        xh[:, h, j * SH : (j + 1) * SH],
                        start=True,
                        stop=True,
                    )
                    nc.vector.tensor_copy(kt_sb[:, j * SH : (j + 1) * SH], kt_ps)

                hz_ps = ps.tile([VW, SH], f32, tag="ps")
                ets = []
                for t in range(NT):
                    sc_ps = ps.tile([128, SH], f32, tag="ps")
                    nc.tensor.matmul(
                        sc_ps,
                        kt_sb[:, t * 128 : (t + 1) * 128],
                        qt_sb,
                        start=True,
                        stop=True,
                    )
                    et = etp.tile([128, SH], bf16, tag="et")
                    nc.scalar.activation(
                        et, sc_ps, mybir.ActivationFunctionType.Exp, scale=SCALE
                    )
                    ets.append(et)
                for t in range(NT):
                    nc.tensor.matmul(
                        hz_ps,
                        vall[:, t * H + h, :],
                        ets[t],
                        start=(t == 0),
                        stop=(t == NT - 1),
                    )
                r_f = sm.tile([1, SH], f32, tag="rf")
                nc.vector.reciprocal(r_f, hz_ps[VW - 1 : VW, :])
                r_b = sm.tile([1, SH], bf16, tag="rb")
                nc.vector.tensor_copy(r_b, r_f)
                br_ps = ps.tile([DK, SH], f32, tag="ps")
                nc.tensor.matmul(br_ps, ones80, r_b, start=True, stop=True)
                br_sb = sm.tile([DK, SH], bf16, tag="brsb")
                nc.vector.tensor_copy(br_sb, br_ps)
                nc.vector.tensor_mul(ct[0:DK, h, :], hz_ps[0:DK, :], br_sb)

            # ---- Stage 5: output projection ----
            ocuts = [(0, 512), (512, 1024), (1024, 1280)]
            for qt in range(SH // 128):
                for o0, o1 in ocuts:
                    op = ps.tile([128, 512], f32, tag="ps")
                    for h in range(H):
                        kh = DK + 1 if h == 15 else DK
                        nc.tensor.matmul(
                            op[:, 0 : o1 - o0],
                            ct[0:kh, h, qt * 128 : (qt + 1) * 128],
                            wo_sb[0:kh, h, o0:o1],
                            start=(h == 0),
                            stop=(h == 15),
                        )
                    o_sb = sm.tile([128, 512], f32, tag="osb")
                    nc.vector.tensor_copy(o_sb[:, 0 : o1 - o0], op[:, 0 : o1 - o0])
                    nc.sync.dma_start(
                        out=out_d[qt * 128 : (qt + 1) * 128, o0:o1],
                        in_=o_sb[:, 0 : o1 - o0],
                    )

    _legalize_waits(nc, mybir)
    return nc


def _host_prep(Wq, bq, Wk, bk, Wv, bv, Wo, bo):
    bf = ml_dtypes.bfloat16
    dprime = np.arange(DK)
    perm = 5 * (dprime % 16) + dprime // 16  # d' -> d

    def aug(Wx, bx):
        # [H, e, d] -> [H, d', e] permuted, + bias row -> [dk+1, H, dk]
        wt = Wx.transpose(0, 2, 1)[:, perm, :]  # [H, d', e]
        a = np.concatenate([wt, bx[:, None, :]], axis=1)  # [H, dk+1, dk]
        return np.ascontiguousarray(a.transpose(1, 0, 2)).astype(bf)

    wq = aug(Wq, bq)
    wk = aug(Wk, bk)
    wv = aug(Wv, bv)

    wo_t = Wo.T.reshape(H, DK, D)  # [h, e, o]
    last = np.zeros((H, 1, D), np.float32)
    last[15, 0, :] = bo
    wo = np.concatenate([wo_t, last], axis=1)  # [H, dk+1, D]
    wo = np.ascontiguousarray(wo.transpose(1, 0, 2)).astype(bf)

    consts = {
        "ident": np.eye(128, dtype=bf),
        "ones2d": np.ones((128, 128), bf),
        "onesrow": np.ones((1, H * S), bf),
        "ones80": np.ones((1, DK), bf),
    }
    return wq, wk, wv, wo, consts


def kernel(**inputs):
    from concourse.bass_utils import run_bass_kernel_spmd

    src = np.asarray(inputs["src"], np.float32)
    wq, wk, wv, wo, consts = _host_prep(
        np.asarray(inputs["Wq"], np.float32),
        np.asarray(inputs["bq"], np.float32),
        np.asarray(inputs["Wk"], np.float32),
        np.asarray(inputs["bk"], np.float32),
        np.asarray(inputs["Wv"], np.float32),
        np.asarray(inputs["bv"], np.float32),
        np.asarray(inputs["Wo"], np.float32),
        np.asarray(inputs["bo"], np.float32),
    )

    if "nc" not in _BUILT:
        _BUILT["nc"] = _build()
    nc = _BUILT["nc"]

    in_maps = []
    for i in range(N_CORES):
        b, qlo = i // 2, (i % 2) * SH
        x = np.roll(src[b], -qlo, axis=0)
        in_maps.append(
            {
                "x": np.ascontiguousarray(x),
                "wq": wq,
                "wk": wk,
                "wv": wv,
                "wo": wo,
                **consts,
            }
        )

    res = run_bass_kernel_spmd(nc, in_maps, core_ids=list(range(N_CORES)))

    out = np.empty((B, S, D), np.float32)
    for i in range(N_CORES):
        b, qlo = i // 2, (i % 2) * SH
        out[b, qlo : qlo + SH] = res.results[i]["out"]
    return out


# revision 8
# speedup vs baseline: 1.3092x; 1.3092x over previous
"""Trainium2 Bass kernel for nn_MultiHeadAttention_60816736911814.

Reference semantics (all derived from `src`; `k`/`v` args ignored):
  x  = channel_shuffle(src)          # [B,S,G,C]->[B,S,C,G] flatten, G=5
  xh = split_heads(x)                # [B,H,S,dk], H=16, dk=80
  q/k/v = per-head Linear(dk,dk)     # weights [H,dk,dk] + bias
  attn  = softmax(q kᵀ / sqrt(dk)) v
  out   = concat(attn) @ Woᵀ + bo    # Wo [D,D], D=1280

Sharding (8 cores, no collectives): core i handles batch b=i//2 and query
rows [512*(i%2), 512*(i%2)+512). Each core gets src[b] ROLLED so its query
rows are rows 0..511 (key order is irrelevant to softmax+sum), letting all
cores run an identical program. Wo is applied per-core on its row slice, so
the full output is a pure concatenation.

All matmuls run in bf16 with fp32 PSUM accumulation. The channel shuffle,
head split and Linear biases are folded into host-side weight layout:
 - device-side xhT rows use d' ordering with d = 5*(d'%16) + d'//16, so the
   channels of head h at row d' are exactly src channel 256*(d'//16)+16h+
   (d'%16) -> a contiguous 16-channel strip per (h, r=d'//16), produced by
   plain 128x128 PE transposes of src + one rectangular SBUF->SBUF DMA.
 - projection weights are permuted with the same d' order and get the bias
   appended as contraction row 80 (paired with a ones row 80 in xhT).
 - softmax denominator Z comes free as row 80 of the attention matmul by
   augmenting V with a ones column.
"""

import numpy as np
import ml_dtypes

B, S, D = 4, 1024, 1280
H, DK, G = 16, 80, 5
N_CORES = 8
SH = S // 2  # 512 query rows per core
SCALE = 1.0 / float(np.sqrt(DK))
NT = S // 128  # 8 s-tiles
NCT = D // 128  # 10 channel tiles

_BUILT = {}


def _legalize_waits(nc, mybir):
    """This walrus build allows 1 sync-wait per instruction (2 on
    EventSemaphore). Tile can emit more; split overflow waits onto
    injected same-engine NoOp carriers placed just before the
    instruction (engines run their stream in order -> AND semantics)."""
    n_fix = 0
    for f in nc.m.functions:
        for blk in f.blocks:
            out = []
            changed = False
            for inst in blk.instructions:
                cap = 2 if type(inst).__name__ == "InstEventSemaphore" else 1
                si = inst.sync_info
                if si is not None and si.on_wait and len(si.on_wait) > cap:
                    waits = list(si.on_wait)
                    for w in waits[:-cap]:
                        nop = mybir.InstNoOp(name=f"I-waitfix-{n_fix}")
                        n_fix += 1
                        nop.engine = inst.engine
                        nop.sync_info = mybir.SyncInfo(on_wait=[w], on_update=[])
                        out.append(nop)
                    inst.sync_info = mybir.SyncInfo(
                        on_wait=waits[-cap:], on_update=list(si.on_update)
                    )
                    changed = True
                out.append(inst)
            if changed:
                try:
                    blk.instructions = out
                except Exception:
                    blk.instructions.clear()
                    blk.instructions.extend(out)
    return n_fix


def _build():
    import concourse.bass as bass
    import concourse.mybir as mybir
    import concourse.tile as tile

    f32 = mybir.dt.float32
    bf16 = mybir.dt.bfloat16

    nc = bass.Bass(trn_type="TRN2", target_bir_lowering=False, debug=False)

    x_d = nc.dram_tensor("x", [S, D], f32, kind="ExternalInput").ap()
    wq_d = nc.dram_tensor("wq", [DK + 1, H, DK], bf16, kind="ExternalInput").ap()
    wk_d = nc.dram_tensor("wk", [DK + 1, H, DK], bf16, kind="ExternalInput").ap()
    wv_d = nc.dram_tensor("wv", [DK + 1, H, DK], bf16, kind="ExternalInput").ap()
    wo_d = nc.dram_tensor("wo", [DK + 1, H, D], bf16, kind="ExternalInput").ap()
    id_d = nc.dram_tensor("ident", [128, 128], bf16, kind="ExternalInput").ap()
    on2_d = nc.dram_tensor("ones2d", [128, 128], bf16, kind="ExternalInput").ap()
    onr_d = nc.dram_tensor("onesrow", [1, H * S], bf16, kind="ExternalInput").ap()
    on80_d = nc.dram_tensor("ones80", [1, DK], bf16, kind="ExternalInput").ap()
    out_d = nc.dram_tensor("out", [SH, D], f32, kind="ExternalOutput").ap()

    with tile.TileContext(nc) as tc:
        with (
            tc.tile_pool(name="const", bufs=1) as const,
            tc.tile_pool(name="big", bufs=1) as big,
            tc.tile_pool(name="ld", bufs=3) as ld,
            tc.tile_pool(name="et", bufs=8) as etp,
            tc.tile_pool(name="sm", bufs=3) as sm,
            tc.tile_pool(name="ps", bufs=7, space="PSUM") as ps,
        ):
            ident = const.tile([128, 128], bf16)
            nc.sync.dma_start(out=ident, in_=id_d)
            ones80 = const.tile([1, DK], bf16)
            nc.sync.dma_start(out=ones80, in_=on80_d)

            wq_sb = big.tile([DK + 1, H, DK], bf16)
            wk_sb = big.tile([DK + 1, H, DK], bf16)
            wv_sb = big.tile([DK + 1, H, DK], bf16)
            wo_sb = big.tile([DK + 1, H, D], bf16)
            nc.sync.dma_start(out=wq_sb, in_=wq_d)
            nc.sync.dma_start(out=wk_sb, in_=wk_d)
            nc.sync.dma_start(out=wv_sb, in_=wv_d)
            nc.sync.dma_start(out=wo_sb, in_=wo_d)

            # XH[d', h, s]: transposed shuffled heads (+ ones row 80)
            xh = big.tile([DK + 1, H, S], bf16)
            nc.sync.dma_start(out=xh[DK : DK + 1, :, :], in_=onr_d)
            # V_ALL[s_in_tile, t*16+h, e(+pad, ones at 96)]
            VW = 97  # Z lands on PSUM partition 96 (32-aligned for engine reads)
            vall = big.tile([128, NT * H, VW], bf16)
            on_bcast = bass.AP(
                tensor=on2_d.tensor,
                offset=0,
                ap=[[128, 128], [0, NT * H], [1, VW - DK]],
            )
            nc.sync.dma_start(out=vall[:, :, DK:VW], in_=on_bcast)
            # concatT[e(+ones), h, q]
            ct = big.tile([DK + 1, H, SH], bf16)
            nc.sync.dma_start(out=ct[DK : DK + 1, 15, :], in_=onr_d[:, 0:SH])

            # ---- Stage 1: load src, cast, transpose, repack ----
            xt = big.tile([128, NCT, S], bf16)  # x transposed [c, ct, s]
            for t in range(NT):
                s_f = ld.tile([128, D], f32)
                nc.sync.dma_start(out=s_f, in_=x_d[t * 128 : (t + 1) * 128, :])
                s_b = ld.tile([128, D], bf16)
                nc.vector.tensor_copy(s_b, s_f)
                for c in range(NCT):
                    p_ps = ps.tile([128, 128], bf16, tag="ps")
                    nc.tensor.transpose(p_ps, s_b[:, c * 128 : (c + 1) * 128], ident)
                    nc.vector.tensor_copy(xt[:, c, t * 128 : (t + 1) * 128], p_ps)
            for h in range(H):
                for r in range(5):
                    c = 2 * r + h // 8
                    poff = 16 * (h % 8)
                    nc.gpsimd.dma_start(
                        out=xh[16 * r : 16 * r + 16, h, :],
                        in_=xt[poff : poff + 16, c, :],
                    )

            # ---- Stage 3: V projections (t-outer, head-grouped) ----
            groups = [list(range(0, 6)), list(range(6, 12)), list(range(12, 16))]
            for t in range(NT):
                for grp in groups:
                    ng = len(grp)
                    vp = ps.tile([128, 6, DK], f32, tag="ps")
                    for i, h in enumerate(grp):
                        nc.tensor.matmul(
                            vp[:, i, :],
                            xh[:, h, t * 128 : (t + 1) * 128],
                            wv_sb[:, h, :],
                            start=True,
                            stop=True,
                        )
                    nc.vector.tensor_copy(
                        vall[:, t * H + grp[0] : t * H + grp[0] + ng, 0:DK],
                        vp[:, 0:ng, :],
                    )

            # ---- Stage 4: projections + attention per head ----
            for h in range(H):
                qt_ps = ps.tile([DK, SH], f32, tag="ps")
                nc.tensor.matmul(
                    qt_ps, wq_sb[:, h, :], xh[:, h, 0:SH], start=True, stop=True
                )
                qt_sb = sm.tile([DK, SH], bf16, tag="qt")
                nc.vector.tensor_copy(qt_sb, qt_ps)
                kt_sb = sm.tile([DK, S], bf16, tag="kt")
                for j in range(2):
                    kt_ps = ps.tile([DK, SH], f32, tag="ps")
                    nc.tensor.matmul(
                        kt_ps,
                        wk_sb[:, h, :],
                        xh[:, h, j * SH : (j + 1) * SH],
                        start=True,
                        stop=True,
                    )
                    nc.vector.tensor_copy(kt_sb[:, j * SH : (j + 1) * SH], kt_ps)

                hz_ps = ps.tile([VW, SH], f32, tag="ps")
                ets = []
                for t in range(NT):
                    sc_ps = ps.tile([128, SH], f32, tag="ps")
                    nc.tensor.matmul(
                        sc_ps,
                        kt_sb[:, t * 128 : (t + 1) * 128],
                        qt_sb,
                        start=True,
                        stop=True,
                    )
                    et = etp.tile([128, SH], bf16, tag="et")
                    nc.scalar.activation(
                        et, sc_ps, mybir.ActivationFunctionType.Exp, scale=SCALE
                    )
                    ets.append(et)
                for t in range(NT):
                    nc.tensor.matmul(
                        hz_ps,
                        vall[:, t * H + h, :],
                        ets[t],
                        start=(t == 0),
                        stop=(t == NT - 1),
                    )
                z_sb = sm.tile([1, SH], bf16, tag="zsb")
                nc.scalar.copy(z_sb, hz_ps[VW - 1 : VW, :])
                br_ps = ps.tile([DK, SH], f32, tag="ps")
                nc.tensor.matmul(br_ps, ones80, z_sb, start=True, stop=True)
                rz = sm.tile([DK, SH], f32, tag="rz")
                nc.vector.reciprocal(rz, br_ps)
                nc.vector.tensor_mul(ct[0:DK, h, :], hz_ps[0:DK, :], rz)

            # ---- Stage 5: output projection ----
            ocuts = [(0, 512), (512, 1024), (1024, 1280)]
            for qt in range(SH // 128):
                for o0, o1 in ocuts:
                    op = ps.tile([128, 512], f32, tag="ps")
                    for h in range(H):
                        kh = DK + 1 if h == 15 else DK
                        nc.tensor.matmul(
                            op[:, 0 : o1 - o0],
                            ct[0:kh, h, qt * 128 : (qt + 1) * 128],
                            wo_sb[0:kh, h, o0:o1],
                            start=(h == 0),
                            stop=(h == 15),
                        )
                    o_sb = sm.tile([128, 512], f32, tag="osb")
                    nc.vector.tensor_copy(o_sb[:, 0 : o1 - o0], op[:, 0 : o1 - o0])
                    nc.gpsimd.dma_start(
                        out=out_d[qt * 128 : (qt + 1) * 128, o0:o1],
                        in_=o_sb[:, 0 : o1 - o0],
                    )

    _legalize_waits(nc, mybir)
    return nc


def _host_prep(Wq, bq, Wk, bk, Wv, bv, Wo, bo):
    bf = ml_dtypes.bfloat16
    dprime = np.arange(DK)
    perm = 5 * (dprime % 16) + dprime // 16  # d' -> d

    def aug(Wx, bx):
        # [H, e, d] -> [H, d', e] permuted, + bias row -> [dk+1, H, dk]
        wt = Wx.transpose(0, 2, 1)[:, perm, :]  # [H, d', e]
        a = np.concatenate([wt, bx[:, None, :]], axis=1)  # [H, dk+1, dk]
        return np.ascontiguousarray(a.transpose(1, 0, 2)).astype(bf)

    wq = aug(Wq, bq)
    wk = aug(Wk, bk)
    wv = aug(Wv, bv)

    wo_t = Wo.T.reshape(H, DK, D)  # [h, e, o]
    last = np.zeros((H, 1, D), np.float32)
    last[15, 0, :] = bo
    wo = np.concatenate([wo_t, last], axis=1)  # [H, dk+1, D]
    wo = np.ascontiguousarray(wo.transpose(1, 0, 2)).astype(bf)

    consts = {
        "ident": np.eye(128, dtype=bf),
        "ones2d": np.ones((128, 128), bf),
        "onesrow": np.ones((1, H * S), bf),
        "ones80": np.ones((1, DK), bf),
    }
    return wq, wk, wv, wo, consts


def kernel(**inputs):
    from concourse.bass_utils import run_bass_kernel_spmd

    src = np.asarray(inputs["src"], np.float32)
    wq, wk, wv, wo, consts = _host_prep(
        np.asarray(inputs["Wq"], np.float32),
        np.asarray(inputs["bq"], np.float32),
        np.asarray(inputs["Wk"], np.float32),
        np.asarray(inputs["bk"], np.float32),
        np.asarray(inputs["Wv"], np.float32),
        np.asarray(inputs["bv"], np.float32),
        np.asarray(inputs["Wo"], np.float32),
        np.asarray(inputs["bo"], np.float32),
    )

    if "nc" not in _BUILT:
        _BUILT["nc"] = _build()
    nc = _BUILT["nc"]

    in_maps = []
    for i in range(N_CORES):
        b, qlo = i // 2, (i % 2) * SH
        x = np.roll(src[b], -qlo, axis=0)
        in_maps.append(
            {
                "x": np.ascontiguousarray(x),
                "wq": wq,
                "wk": wk,
                "wv": wv,
                "wo": wo,
                **consts,
            }
        )

    res = run_bass_kernel_spmd(nc, in_maps, core_ids=list(range(N_CORES)))

    out = np.empty((B, S, D), np.float32)
    for i in range(N_CORES):
        b, qlo = i // 2, (i % 2) * SH
        out[b, qlo : qlo + SH] = res.results[i]["out"]
    return out


# revision 11
# speedup vs baseline: 1.6126x; 1.2318x over previous
"""Trainium2 Bass kernel for nn_MultiHeadAttention_60816736911814.

Reference semantics (all derived from `src`; `k`/`v` args ignored):
  x  = channel_shuffle(src)          # [B,S,G,C]->[B,S,C,G] flatten, G=5
  xh = split_heads(x)                # [B,H,S,dk], H=16, dk=80
  q/k/v = per-head Linear(dk,dk)     # weights [H,dk,dk] + bias
  attn  = softmax(q kᵀ / sqrt(dk)) v
  out   = concat(attn) @ Woᵀ + bo    # Wo [D,D], D=1280

Sharding (8 cores, no collectives): core i handles batch b=i//2 and query
rows [512*(i%2), 512*(i%2)+512). Each core gets src[b] ROLLED so its query
rows are rows 0..511 (key order is irrelevant to softmax+sum), letting all
cores run an identical program. Wo is applied per-core on its row slice, so
the full output is a pure concatenation.

All matmuls run in bf16 with fp32 PSUM accumulation. The channel shuffle,
head split and Linear biases are folded into host-side weight layout:
 - device-side xhT rows use d' ordering with d = 5*(d'%16) + d'//16, so the
   channels of head h at row d' are exactly src channel 256*(d'//16)+16h+
   (d'%16) -> a contiguous 16-channel strip per (h, r=d'//16), produced by
   plain 128x128 PE transposes of src + one rectangular SBUF->SBUF DMA.
 - projection weights are permuted with the same d' order and get the bias
   appended as contraction row 80 (paired with a ones row 80 in xhT).
 - softmax denominator Z comes free as row 80 of the attention matmul by
   augmenting V with a ones column.
"""

import numpy as np
import ml_dtypes

B, S, D = 4, 1024, 1280
H, DK, G = 16, 80, 5
N_CORES = 8
SH = S // 2  # 512 query rows per core
SCALE = 1.0 / float(np.sqrt(DK))
NT = S // 128  # 8 s-tiles
NCT = D // 128  # 10 channel tiles

_BUILT = {}


def _legalize_waits(nc, mybir):
    """This walrus build allows 1 sync-wait per instruction (2 on
    EventSemaphore). Tile can emit more; split overflow waits onto
    injected same-engine NoOp carriers placed just before the
    instruction (engines run their stream in order -> AND semantics)."""
    n_fix = 0
    for f in nc.m.functions:
        for blk in f.blocks:
            out = []
            changed = False
            for inst in blk.instructions:
                cap = 2 if type(inst).__name__ == "InstEventSemaphore" else 1
                si = inst.sync_info
                if si is not None and si.on_wait and len(si.on_wait) > cap:
                    waits = list(si.on_wait)
                    for w in waits[:-cap]:
                        nop = mybir.InstNoOp(name=f"I-waitfix-{n_fix}")
                        n_fix += 1
                        nop.engine = inst.engine
                        nop.sync_info = mybir.SyncInfo(on_wait=[w], on_update=[])
                        out.append(nop)
                    inst.sync_info = mybir.SyncInfo(
                        on_wait=waits[-cap:], on_update=list(si.on_update)
                    )
                    changed = True
                out.append(inst)
            if changed:
                try:
                    blk.instructions = out
                except Exception:
                    blk.instructions.clear()
                    blk.instructions.extend(out)
    return n_fix


def _build():
    import concourse.bass as bass
    import concourse.mybir as mybir
    import concourse.tile as tile

    f32 = mybir.dt.float32
    bf16 = mybir.dt.bfloat16

    nc = bass.Bass(trn_type="TRN2", target_bir_lowering=False, debug=False)

    x_d = nc.dram_tensor("x", [S, D], f32, kind="ExternalInput").ap()
    wq_d = nc.dram_tensor("wq", [DK + 1, H, DK], bf16, kind="ExternalInput").ap()
    wk_d = nc.dram_tensor("wk", [DK + 1, H, DK], bf16, kind="ExternalInput").ap()
    wv_d = nc.dram_tensor("wv", [DK + 1, H, DK], bf16, kind="ExternalInput").ap()
    wo_d = nc.dram_tensor("wo", [DK + 1, H, D], bf16, kind="ExternalInput").ap()
    id_d = nc.dram_tensor("ident", [128, 128], bf16, kind="ExternalInput").ap()
    on2_d = nc.dram_tensor("ones2d", [128, 128], bf16, kind="ExternalInput").ap()
    onr_d = nc.dram_tensor("onesrow", [1, H * S], bf16, kind="ExternalInput").ap()
    on80_d = nc.dram_tensor("ones80", [1, DK], bf16, kind="ExternalInput").ap()
    out_d = nc.dram_tensor("out", [SH, D], f32, kind="ExternalOutput").ap()

    with tile.TileContext(nc) as tc:
        with (
            tc.tile_pool(name="const", bufs=1) as const,
            tc.tile_pool(name="big", bufs=1) as big,
            tc.tile_pool(name="ld", bufs=3) as ld,
            tc.tile_pool(name="et", bufs=8) as etp,
            tc.tile_pool(name="sm", bufs=3) as sm,
            tc.tile_pool(name="ps", bufs=4, space="PSUM") as ps,
        ):
            ident = const.tile([128, 128], bf16)
            nc.scalar.dma_start(out=ident, in_=id_d)
            ones80 = const.tile([1, DK], bf16)
            nc.scalar.dma_start(out=ones80, in_=on80_d)

            wq_sb = big.tile([DK + 1, H, DK], bf16)
            wk_sb = big.tile([DK + 1, H, DK], bf16)
            wv_sb = big.tile([DK + 1, H, DK], bf16)
            wo_sb = big.tile([DK + 1, H, D], bf16)
            nc.scalar.dma_start(out=wq_sb, in_=wq_d)
            nc.scalar.dma_start(out=wk_sb, in_=wk_d)
            nc.scalar.dma_start(out=wv_sb, in_=wv_d)
            nc.scalar.dma_start(out=wo_sb, in_=wo_d)

            # XH[d', h, s]: transposed shuffled heads (+ ones row 80)
            xh = big.tile([DK + 1, H, S], bf16)
            nc.scalar.dma_start(out=xh[DK : DK + 1, :, :], in_=onr_d)
            # V_ALL[s_in_tile, t*16+h, e(+pad, ones at 96)]
            VW = 97  # Z lands on PSUM partition 96 (32-aligned for engine reads)
            vall = big.tile([128, NT * H, VW], bf16)
            nc.gpsimd.memset(vall[:, :, DK:VW], 1.0)
            # concatT[e(+ones), h, q]
            ct = big.tile([DK + 1, H, SH], bf16)
            nc.scalar.dma_start(out=ct[DK : DK + 1, 15, :], in_=onr_d[:, 0:SH])

            # ---- Stage 1: load src, cast, transpose, repack ----
            xt = big.tile([128, NCT, S], bf16)  # x transposed [c, ct, s]
            for t in range(NT):
                s_f = ld.tile([128, D], f32)
                nc.sync.dma_start(out=s_f, in_=x_d[t * 128 : (t + 1) * 128, :])
                s_b = ld.tile([128, D], bf16)
                nc.vector.tensor_copy(s_b, s_f)
                for c in range(NCT):
                    p_ps = ps.tile([128, 128], bf16, tag="rot", bufs=3)
                    nc.tensor.transpose(p_ps, s_b[:, c * 128 : (c + 1) * 128], ident)
                    nc.vector.tensor_copy(xt[:, c, t * 128 : (t + 1) * 128], p_ps)
            for h in range(H):
                for r in range(5):
                    c = 2 * r + h // 8
                    poff = 16 * (h % 8)
                    nc.gpsimd.dma_start(
                        out=xh[16 * r : 16 * r + 16, h, :],
                        in_=xt[poff : poff + 16, c, :],
                    )

            # ---- Stage 3: V projections (t-outer, head-grouped) ----
            groups = [list(range(0, 6)), list(range(6, 12)), list(range(12, 16))]
            for t in range(NT):
                for grp in groups:
                    ng = len(grp)
                    vp = ps.tile([128, 6, DK], f32, tag="rot", bufs=3)
                    for i, h in enumerate(grp):
                        nc.tensor.matmul(
                            vp[:, i, :],
                            xh[:, h, t * 128 : (t + 1) * 128],
                            wv_sb[:, h, :],
                            start=True,
                            stop=True,
                        )
                    nc.vector.tensor_copy(
                        vall[:, t * H + grp[0] : t * H + grp[0] + ng, 0:DK],
                        vp[:, 0:ng, :],
                    )

            # ---- Stage 4: projections + attention per head ----
            for h in range(H):
                qt_ps = ps.tile([DK, SH], f32, tag="qk", bufs=2)
                nc.tensor.matmul(
                    qt_ps, wq_sb[:, h, :], xh[:, h, 0:SH], start=True, stop=True
                )
                qt_sb = sm.tile([DK, SH], bf16, tag="qt")
                nc.vector.tensor_copy(qt_sb, qt_ps)
                kt_sb = sm.tile([DK, S], bf16, tag="kt")
                for j in range(2):
                    kt_ps = ps.tile([DK, SH], f32, tag="qk", bufs=2)
                    nc.tensor.matmul(
                        kt_ps,
                        wk_sb[:, h, :],
                        xh[:, h, j * SH : (j + 1) * SH],
                        start=True,
                        stop=True,
                    )
                    nc.vector.tensor_copy(kt_sb[:, j * SH : (j + 1) * SH], kt_ps)

                hz_ps = ps.tile([VW, SH], f32, tag="hz", bufs=2)
                ets = []
                for t in range(NT):
                    sc_ps = ps.tile([128, SH], f32, tag="rot", bufs=3)
                    nc.tensor.matmul(
                        sc_ps,
                        kt_sb[:, t * 128 : (t + 1) * 128],
                        qt_sb,
                        start=True,
                        stop=True,
                    )
                    et = etp.tile([128, SH], bf16, tag="et")
                    nc.scalar.activation(
                        et, sc_ps, mybir.ActivationFunctionType.Exp, scale=SCALE
                    )
                    ets.append(et)
                for t in range(NT):
                    nc.tensor.matmul(
                        hz_ps,
                        vall[:, t * H + h, :],
                        ets[t],
                        start=(t == 0),
                        stop=(t == NT - 1),
                    )
                z_sb = sm.tile([1, SH], bf16, tag="zsb")
                nc.scalar.copy(z_sb, hz_ps[VW - 1 : VW, :])
                br_ps = ps.tile([DK, SH], f32, tag="br", bufs=1)
                nc.tensor.matmul(br_ps, ones80, z_sb, start=True, stop=True)
                rz = sm.tile([DK, SH], f32, tag="rz")
                nc.vector.reciprocal(rz, br_ps)
                nc.vector.tensor_mul(ct[0:DK, h, :], hz_ps[0:DK, :], rz)

            # ---- Stage 5: output projection ----
            ocuts = [(0, 512), (512, 1024), (1024, 1280)]
            for qt in range(SH // 128):
                for o0, o1 in ocuts:
                    op = ps.tile([128, 512], f32, tag="rot", bufs=3)
                    for h in range(H):
                        kh = DK + 1 if h == 15 else DK
                        nc.tensor.matmul(
                            op[:, 0 : o1 - o0],
                            ct[0:kh, h, qt * 128 : (qt + 1) * 128],
                            wo_sb[0:kh, h, o0:o1],
                            start=(h == 0),
                            stop=(h == 15),
                        )
                    o_sb = sm.tile([128, 512], f32, tag="osb")
                    nc.vector.tensor_copy(o_sb[:, 0 : o1 - o0], op[:, 0 : o1 - o0])
                    nc.gpsimd.dma_start(
                        out=out_d[qt * 128 : (qt + 1) * 128, o0:o1],
                        in_=o_sb[:, 0 : o1 - o0],
                    )

    _legalize_waits(nc, mybir)
    return nc


def _host_prep(Wq, bq, Wk, bk, Wv, bv, Wo, bo):
    bf = ml_dtypes.bfloat16
    dprime = np.arange(DK)
    perm = 5 * (dprime % 16) + dprime // 16  # d' -> d

    def aug(Wx, bx):
        # [H, e, d] -> [H, d', e] permuted, + bias row -> [dk+1, H, dk]
        wt = Wx.transpose(0, 2, 1)[:, perm, :]  # [H, d', e]
        a = np.concatenate([wt, bx[:, None, :]], axis=1)  # [H, dk+1, dk]
        return np.ascontiguousarray(a.transpose(1, 0, 2)).astype(bf)

    wq = aug(Wq, bq)
    wk = aug(Wk, bk)
    wv = aug(Wv, bv)

    wo_t = Wo.T.reshape(H, DK, D)  # [h, e, o]
    last = np.zeros((H, 1, D), np.float32)
    last[15, 0, :] = bo
    wo = np.concatenate([wo_t, last], axis=1)  # [H, dk+1, D]
    wo = np.ascontiguousarray(wo.transpose(1, 0, 2)).astype(bf)

    consts = {
        "ident": np.eye(128, dtype=bf),
        "ones2d": np.ones((128, 128), bf),
        "onesrow": np.ones((1, H * S), bf),
        "ones80": np.ones((1, DK), bf),
    }
    return wq, wk, wv, wo, consts


def kernel(**inputs):
    from concourse.bass_utils import run_bass_kernel_spmd

    src = np.asarray(inputs["src"], np.float32)
    wq, wk, wv, wo, consts = _host_prep(
        np.asarray(inputs["Wq"], np.float32),
        np.asarray(inputs["bq"], np.float32),
        np.asarray(inputs["Wk"], np.float32),
        np.asarray(inputs["bk"], np.float32),
        np.asarray(inputs["Wv"], np.float32),
        np.asarray(inputs["bv"], np.float32),
        np.asarray(inputs["Wo"], np.float32),
        np.asarray(inputs["bo"], np.float32),
    )

    if "nc" not in _BUILT:
        _BUILT["nc"] = _build()
    nc = _BUILT["nc"]

    in_maps = []
    for i in range(N_CORES):
        b, qlo = i // 2, (i % 2) * SH
        x = np.roll(src[b], -qlo, axis=0)
        in_maps.append(
            {
                "x": np.ascontiguousarray(x),
                "wq": wq,
                "wk": wk,
                "wv": wv,
                "wo": wo,
                **consts,
            }
        )

    res = run_bass_kernel_spmd(nc, in_maps, core_ids=list(range(N_CORES)))

    out = np.empty((B, S, D), np.float32)
    for i in range(N_CORES):
        b, qlo = i // 2, (i % 2) * SH
        out[b, qlo : qlo + SH] = res.results[i]["out"]
    return out


# revision 14
# speedup vs baseline: 1.8805x; 1.1661x over previous
"""Trainium2 Bass kernel for nn_MultiHeadAttention_60816736911814.

Reference semantics (all derived from `src`; `k`/`v` args ignored):
  x  = channel_shuffle(src)          # [B,S,G,C]->[B,S,C,G] flatten, G=5
  xh = split_heads(x)                # [B,H,S,dk], H=16, dk=80
  q/k/v = per-head Linear(dk,dk)     # weights [H,dk,dk] + bias
  attn  = softmax(q kᵀ / sqrt(dk)) v
  out   = concat(attn) @ Woᵀ + bo    # Wo [D,D], D=1280

Sharding (8 cores, no collectives): core i handles batch b=i//2 and query
rows [512*(i%2), 512*(i%2)+512). Each core gets src[b] ROLLED so its query
rows are rows 0..511 (key order is irrelevant to softmax+sum), letting all
cores run an identical program. Wo is applied per-core on its row slice, so
the full output is a pure concatenation.

All matmuls run in bf16 with fp32 PSUM accumulation. The channel shuffle,
head split and Linear biases are folded into host-side weight layout:
 - device-side xhT rows use d' ordering with d = 5*(d'%16) + d'//16, so the
   channels of head h at row d' are exactly src channel 256*(d'//16)+16h+
   (d'%16) -> a contiguous 16-channel strip per (h, r=d'//16), produced by
   plain 128x128 PE transposes of src + one rectangular SBUF->SBUF DMA.
 - projection weights are permuted with the same d' order and get the bias
   appended as contraction row 80 (paired with a ones row 80 in xhT).
 - softmax denominator Z comes free as row 80 of the attention matmul by
   augmenting V with a ones column.
"""

import numpy as np
import ml_dtypes

B, S, D = 4, 1024, 1280
H, DK, G = 16, 80, 5
N_CORES = 8
SH = S // 2  # 512 query rows per core
SCALE = 1.0 / float(np.sqrt(DK))
NT = S // 128  # 8 s-tiles
NCT = D // 128  # 10 channel tiles

_BUILT = {}


def _legalize_waits(nc, mybir):
    """This walrus build allows 1 sync-wait per instruction (2 on
    EventSemaphore). Tile can emit more; split overflow waits onto
    injected same-engine NoOp carriers placed just before the
    instruction (engines run their stream in order -> AND semantics)."""
    n_fix = 0
    for f in nc.m.functions:
        for blk in f.blocks:
            out = []
            changed = False
            for inst in blk.instructions:
                cap = 2 if type(inst).__name__ == "InstEventSemaphore" else 1
                si = inst.sync_info
                if si is not None and si.on_wait and len(si.on_wait) > cap:
                    waits = list(si.on_wait)
                    for w in waits[:-cap]:
                        nop = mybir.InstNoOp(name=f"I-waitfix-{n_fix}")
                        n_fix += 1
                        nop.engine = inst.engine
                        nop.sync_info = mybir.SyncInfo(on_wait=[w], on_update=[])
                        out.append(nop)
                    inst.sync_info = mybir.SyncInfo(
                        on_wait=waits[-cap:], on_update=list(si.on_update)
                    )
                    changed = True
                out.append(inst)
            if changed:
                try:
                    blk.instructions = out
                except Exception:
                    blk.instructions.clear()
                    blk.instructions.extend(out)
    return n_fix


def _build(legalize=True):
    import concourse.bass as bass
    import concourse.mybir as mybir
    import concourse.tile as tile

    f32 = mybir.dt.float32
    bf16 = mybir.dt.bfloat16

    nc = bass.Bass(trn_type="TRN2", target_bir_lowering=False, debug=False)

    x_d = nc.dram_tensor("x", [S, D], f32, kind="ExternalInput").ap()
    wq_d = nc.dram_tensor("wq", [DK + 1, H, DK], bf16, kind="ExternalInput").ap()
    wk_d = nc.dram_tensor("wk", [DK + 1, H, DK], bf16, kind="ExternalInput").ap()
    wv_d = nc.dram_tensor("wv", [DK + 1, H, DK], bf16, kind="ExternalInput").ap()
    wo_d = nc.dram_tensor("wo", [128, 11, D], bf16, kind="ExternalInput").ap()
    id_d = nc.dram_tensor("ident", [128, 128], bf16, kind="ExternalInput").ap()
    on2_d = nc.dram_tensor("ones2d", [128, 128], bf16, kind="ExternalInput").ap()
    onr_d = nc.dram_tensor("onesrow", [1, H * S], bf16, kind="ExternalInput").ap()
    on80_d = nc.dram_tensor("ones80", [1, DK], bf16, kind="ExternalInput").ap()
    out_d = nc.dram_tensor("out", [SH, D], f32, kind="ExternalOutput").ap()

    with tile.TileContext(nc) as tc:
        with (
            tc.tile_pool(name="const", bufs=1) as const,
            tc.tile_pool(name="big", bufs=1) as big,
            tc.tile_pool(name="ld", bufs=3) as ld,
            tc.tile_pool(name="et", bufs=8) as etp,
            tc.tile_pool(name="sm", bufs=3) as sm,
            tc.tile_pool(name="ps", bufs=4, space="PSUM") as ps,
        ):
            ident = const.tile([128, 128], bf16)
            nc.scalar.dma_start(out=ident, in_=id_d)
            ones80 = const.tile([1, DK], bf16)
            nc.scalar.dma_start(out=ones80, in_=on80_d)

            wq_sb = big.tile([DK + 1, H, DK], bf16)
            wk_sb = big.tile([DK + 1, H, DK], bf16)
            wv_sb = big.tile([DK + 1, H, DK], bf16)
            wo_sb = big.tile([128, 11, D], bf16)
            nc.scalar.dma_start(out=wq_sb, in_=wq_d)
            nc.scalar.dma_start(out=wk_sb, in_=wk_d)
            nc.scalar.dma_start(out=wv_sb, in_=wv_d)
            nc.scalar.dma_start(out=wo_sb, in_=wo_d)

            # XH[d', h, s]: transposed shuffled heads (+ ones row 80)
            xh = big.tile([DK + 1, H, S], bf16)
            nc.scalar.dma_start(out=xh[DK : DK + 1, :, :], in_=onr_d)
            # V_ALL[s_in_tile, t*16+h, e(+pad, ones at 96)]
            VW = 97  # Z lands on PSUM partition 96 (32-aligned for engine reads)
            vall = big.tile([128, NT * H, VW], bf16)
            nc.gpsimd.memset(vall[:, :, DK:VW], 1.0)
            # concatT[e, h, q] and K=128-packed ctp[j%128, j//128, q]
            ct = big.tile([DK + 1, H, SH], bf16)
            ctp = big.tile([128, 11, SH], bf16)
            nc.scalar.dma_start(out=ctp[0:1, 10, :], in_=onr_d[:, 0:SH])

            # ---- Stage 1: load src, cast, transpose (c-outer), repack ----
            xt = big.tile([128, NCT, S], bf16)  # x transposed [c, ct, s]
            sbs = []
            for t in range(NT):
                s_f = ld.tile([128, D], f32, tag="sf", bufs=3)
                nc.sync.dma_start(out=s_f, in_=x_d[t * 128 : (t + 1) * 128, :])
                s_b = ld.tile([128, D], bf16, tag="sb", bufs=NT)
                nc.vector.tensor_copy(s_b, s_f)
                sbs.append(s_b)
            rep = 0
            for c in [0, 2, 4, 6, 8, 1, 3, 5, 7, 9]:
                for t in range(NT):
                    p_ps = ps.tile([128, 128], bf16, tag="rot", bufs=3)
                    nc.tensor.transpose(p_ps, sbs[t][:, c * 128 : (c + 1) * 128], ident)
                    nc.vector.tensor_copy(xt[:, c, t * 128 : (t + 1) * 128], p_ps)
                r = c // 2
                for h in range(8 * (c % 2), 8 * (c % 2) + 8):
                    poff = 16 * (h % 8)
                    rep += 1
                    nc.gpsimd.dma_start(
                        out=xh[16 * r : 16 * r + 16, h, :],
                        in_=xt[poff : poff + 16, c, :],
                    )

            # ---- Stage 3: V projections (t-outer, head-grouped) ----
            groups = [list(range(0, 6)), list(range(6, 12)), list(range(12, 16))]
            for t in range(NT):
                for grp in groups:
                    ng = len(grp)
                    vp = ps.tile([128, 6, DK], f32, tag="rot", bufs=3)
                    for i, h in enumerate(grp):
                        nc.tensor.matmul(
                            vp[:, i, :],
                            xh[:, h, t * 128 : (t + 1) * 128],
                            wv_sb[:, h, :],
                            start=True,
                            stop=True,
                        )
                    nc.vector.tensor_copy(
                        vall[:, t * H + grp[0] : t * H + grp[0] + ng, 0:DK],
                        vp[:, 0:ng, :],
                    )

            # ---- Stage 4: projections + attention per head ----
            for h in range(H):
                qt_ps = ps.tile([DK, SH], f32, tag="qk", bufs=2)
                nc.tensor.matmul(
                    qt_ps, wq_sb[:, h, :], xh[:, h, 0:SH], start=True, stop=True
                )
                qt_sb = sm.tile([DK, SH], bf16, tag="qt")
                nc.vector.tensor_copy(qt_sb, qt_ps)
                kt_sb = sm.tile([DK, S], bf16, tag="kt")
                for j in range(2):
                    kt_ps = ps.tile([DK, SH], f32, tag="qk", bufs=2)
                    nc.tensor.matmul(
                        kt_ps,
                        wk_sb[:, h, :],
                        xh[:, h, j * SH : (j + 1) * SH],
                        start=True,
                        stop=True,
                    )
                    nc.vector.tensor_copy(kt_sb[:, j * SH : (j + 1) * SH], kt_ps)

                hz_ps = ps.tile([VW, SH], f32, tag="hz", bufs=2)
                ets = []
                for t in range(NT):
                    sc_ps = ps.tile([128, SH], f32, tag="rot", bufs=3)
                    nc.tensor.matmul(
                        sc_ps,
                        kt_sb[:, t * 128 : (t + 1) * 128],
                        qt_sb,
                        start=True,
                        stop=True,
                    )
                    et = etp.tile([128, SH], bf16, tag="et")
                    nc.scalar.activation(
                        et, sc_ps, mybir.ActivationFunctionType.Exp, scale=SCALE
                    )
                    ets.append(et)
                for t in range(NT):
                    nc.tensor.matmul(
                        hz_ps,
                        vall[:, t * H + h, :],
                        ets[t],
                        start=(t == 0),
                        stop=(t == NT - 1),
                    )
                z_sb = sm.tile([1, SH], bf16, tag="zsb")
                nc.scalar.copy(z_sb, hz_ps[VW - 1 : VW, :])
                br_ps = ps.tile([DK, SH], f32, tag="br", bufs=1)
                nc.tensor.matmul(br_ps, ones80, z_sb, start=True, stop=True)
                rz = sm.tile([DK, SH], f32, tag="rz")
                nc.vector.reciprocal(rz, br_ps)
                nc.vector.tensor_mul(ct[0:DK, h, :], hz_ps[0:DK, :], rz)
                j0 = DK * h
                pl, off = j0 // 128, j0 % 128
                l1 = min(128 - off, DK)
                nc.gpsimd.dma_start(
                    out=ctp[off : off + l1, pl, :], in_=ct[0:l1, h, :]
                )
                if l1 < DK:
                    nc.sync.dma_start(
                        out=ctp[0 : DK - l1, pl + 1, :], in_=ct[l1:DK, h, :]
                    )

            # ---- Stage 5: output projection ----
            ocuts = [(0, 512), (512, 1024), (1024, 1280)]
            for qt in range(SH // 128):
                for o0, o1 in ocuts:
                    op = ps.tile([128, 512], f32, tag="rot", bufs=3)
                    for jt in range(11):
                        kh = 1 if jt == 10 else 128
                        nc.tensor.matmul(
                            op[:, 0 : o1 - o0],
                            ctp[0:kh, jt, qt * 128 : (qt + 1) * 128],
                            wo_sb[0:kh, jt, o0:o1],
                            start=(jt == 0),
                            stop=(jt == 10),
                        )
                    o_sb = sm.tile([128, 512], f32, tag="osb")
                    nc.vector.tensor_copy(o_sb[:, 0 : o1 - o0], op[:, 0 : o1 - o0])
                    nc.gpsimd.dma_start(
                        out=out_d[qt * 128 : (qt + 1) * 128, o0:o1],
                        in_=o_sb[:, 0 : o1 - o0],
                    )

    if legalize:
        _legalize_waits(nc, mybir)
    return nc


def _host_prep(Wq, bq, Wk, bk, Wv, bv, Wo, bo):
    bf = ml_dtypes.bfloat16
    dprime = np.arange(DK)
    perm = 5 * (dprime % 16) + dprime // 16  # d' -> d

    def aug(Wx, bx):
        # [H, e, d] -> [H, d', e] permuted, + bias row -> [dk+1, H, dk]
        wt = Wx.transpose(0, 2, 1)[:, perm, :]  # [H, d', e]
        a = np.concatenate([wt, bx[:, None, :]], axis=1)  # [H, dk+1, dk]
        return np.ascontiguousarray(a.transpose(1, 0, 2)).astype(bf)

    wq = aug(Wq, bq)
    wk = aug(Wk, bk)
    wv = aug(Wv, bv)

    wo_t = np.concatenate([Wo.T, np.zeros((128 * 11 - D, D), np.float32)])
    wo_t[D] = bo  # row 0 of plane 10, paired with the ones row in ctp
    wo = np.ascontiguousarray(
        wo_t.reshape(11, 128, D).transpose(1, 0, 2)
    ).astype(bf)

    consts = {
        "ident": np.eye(128, dtype=bf),
        "ones2d": np.ones((128, 128), bf),
        "onesrow": np.ones((1, H * S), bf),
        "ones80": np.ones((1, DK), bf),
    }
    return wq, wk, wv, wo, consts


def kernel(**inputs):
    from concourse.bass_utils import run_bass_kernel_spmd

    src = np.asarray(inputs["src"], np.float32)
    wq, wk, wv, wo, consts = _host_prep(
        np.asarray(inputs["Wq"], np.float32),
        np.asarray(inputs["bq"], np.float32),
        np.asarray(inputs["Wk"], np.float32),
        np.asarray(inputs["bk"], np.float32),
        np.asarray(inputs["Wv"], np.float32),
        np.asarray(inputs["bv"], np.float32),
        np.asarray(inputs["Wo"], np.float32),
        np.asarray(inputs["bo"], np.float32),
    )

    if "nc" not in _BUILT:
        _BUILT["nc"] = _build()
    nc = _BUILT["nc"]

    in_maps = []
    for i in range(N_CORES):
        b, qlo = i // 2, (i % 2) * SH
        x = np.roll(src[b], -qlo, axis=0)
        in_maps.append(
            {
                "x": np.ascontiguousarray(x),
                "wq": wq,
                "wk": wk,
                "wv": wv,
                "wo": wo,
                **consts,
            }
        )

    res = run_bass_kernel_spmd(nc, in_maps, core_ids=list(range(N_CORES)))

    out = np.empty((B, S, D), np.float32)
    for i in range(N_CORES):
        b, qlo = i // 2, (i % 2) * SH
        out[b, qlo : qlo + SH] = res.results[i]["out"]
    return out
